# revision 23
# baseline (speedup 1.0000x reference)
"""Capsule-routing kernel for Trainium2, data-parallel over batch (8 cores).

Math: the reference's per-instance routing (unique -> gather -> attention)
is reformulated as a dense masked softmax over the 64x64 cell grid:
  - all per-cell quantities (attention keys, value-scalar, activation logit)
    come from one fused per-image GEMM,
  - the relative-position encoding's mean term cancels in the softmax and
    reduces to a rank-1 correction computed from per-instance occupancy sums,
  - per-instance dedup of points is a scatter of ones into a cell bitmap,
  - all 32 instances reduce in a single accumulated PE matmul against the
    occupancy mask.

Perf structure (v2):
  - X is converted to fp16 host-side and packed so each cell-chunk's ten
    128-channel slices are one contiguous [128, 10*S] DMA (halves HBM
    traffic, the roofline term, and needs one HWDGE op per chunk),
  - chunk sizes taper [512x7, 256, 128, 128] so the post-GEMM chain of the
    last chunk (the serial tail after the final X byte) is short,
  - all small input DMAs ride the ACT HWDGE ring, X rides the SP ring,
    nothing serializes on GPSIMD descriptor generation,
  - the occupancy scatter pipeline is emitted first so occt is ready by the
    time PE reaches the deferred first psum3 accumulation block,
  - at-tiles pool is deep (36) so psum3 consumption can never backpressure
    the X stream.
"""
import sys

sys.path.insert(0, "/opt/trn_rl_repo")

import numpy as np

import concourse.bacc as bacc
import concourse.mybir as mybir
from concourse import masks, tile
from concourse.bass_utils import run_bass_kernel_spmd

F32 = mybir.dt.float32
F16 = mybir.dt.float16
BF16 = mybir.dt.bfloat16
I32 = mybir.dt.int32
I16 = mybir.dt.int16

B = 8
CIN = 1280
NCELL = 4096  # 64x64 feature grid
NCAPS = 19
NI = 32  # instances per image
NPTS = 256  # points per instance
DK = 64
EPS = 1e-6
NCH = 10  # channel chunks of 128
NCK = 32  # 128-cell subchunks
CHUNKS = [512] * 7 + [384, 128]  # cell chunk sizes, sum = 4096
XCOLS = 10 * NCELL  # packed X16 columns
PSO_AFTER = 3  # emit occ transposes after this chunk's GEMM
PS3_AFTER = 4  # start draining psum3 accumulation backlog at this chunk

_CACHE = {}

# Force every activation onto the one table set that covers exp/ln/copy so
# the ACT engine never reloads its function tables mid-kernel.
_ONE_SET = "natural_log_exp_and_others"
_orig_get_tables = None


def _patched_tables(arch):
    full = _orig_get_tables(arch)
    return {
        name: (funcs if name == _ONE_SET else set())
        for name, funcs in full.items()
    }


def _install_act_table_patch():
    global _orig_get_tables
    if _orig_get_tables is None:
        _orig_get_tables = bacc.get_activation_tables
        bacc.get_activation_tables = _patched_tables


def _build_nc(dbg=False, loop_n=1, mode="full"):
    key = ("nc", dbg, loop_n, mode)
    if key in _CACHE:
        return _CACHE[key]

    _install_act_table_patch()
    nc = bacc.Bacc(None, target_bir_lowering=False, debug=False)

    X16 = nc.dram_tensor("X16", [128, XCOLS], F16, kind="ExternalInput")
    W16 = nc.dram_tensor("W16", [128, 726], F16, kind="ExternalInput")
    QT8 = nc.dram_tensor("QT8", [DK, NCAPS], F32, kind="ExternalInput")
    WC2 = nc.dram_tensor("WC2", [128, 2 * NCK], F32, kind="ExternalInput")
    PTS = nc.dram_tensor("PTS", [NI, 2 * NPTS], I32, kind="ExternalInput")
    C316 = nc.dram_tensor("C316", [3, NCELL], F16, kind="ExternalInput")
    OUT = nc.dram_tensor("OUT", [NI, NCAPS], F32, kind="ExternalOutput")

    with tile.TileContext(nc) as tc:
        with (
            tc.tile_pool(name="const", bufs=1) as cpool,
            tc.tile_pool(name="xp", bufs=6) as xpool,
            tc.tile_pool(name="m1", bufs=1) as m1pool,
            tc.tile_pool(name="small", bufs=1) as spool,
            tc.tile_pool(name="ap", bufs=36) as apool,
            tc.tile_pool(name="ps1", bufs=3, space="PSUM") as ps1,
            tc.tile_pool(name="pst", bufs=1, space="PSUM") as pst,
            tc.tile_pool(name="ps2", bufs=2, space="PSUM") as ps2,
            tc.tile_pool(name="pso", bufs=1, space="PSUM") as pso,
            tc.tile_pool(name="ps3", bufs=1, space="PSUM") as ps3,
        ):
            # ---- constants ----
            id128 = cpool.tile([128, 128], F32)
            masks.make_identity(nc, id128[:])
            id32b = cpool.tile([32, 32], BF16)
            masks.make_identity(nc, id32b[:])

            # ---- small input DMAs (SP HWDGE ring). ptsb/wsb go ahead of
            # the X stream (needed first); the rest are issued inside the
            # chunk loop after X chunk 0 so their HWDGE gen hides under it.
            ptsb = spool.tile([NI, 2 * NPTS], I32)
            nc.scalar.dma_start(ptsb[:], PTS[:])  # ACT ring, parallel HWDGE
            wsb = cpool.tile([128, 726], F16)
            nc.sync.dma_start(wsb[:], W16[:])
            qsb = cpool.tile([DK, NCAPS], F32)
            wcsb = cpool.tile([128, 2 * NCK], F32)
            c3sb = cpool.tile([3, NCELL], F16)

            def _small_dmas():
                nc.sync.dma_start(c3sb[:], C316[:])
                nc.sync.dma_start(qsb[:], QT8[:])
                nc.sync.dma_start(wcsb[:], WC2[:])

            xres = cpool.tile([128, 5120], F16)
            if mode == "compute":
                nc.sync.dma_start(xres[:], X16[:, 0:5120])

            def _dma_body():
                col = 0
                for S in CHUNKS:
                    xt = xpool.tile([128, 5120], F16, tag="xt")
                    nc.sync.dma_start(xt[:, 0 : 10 * S], X16[:, col : col + 10 * S])
                    col += 10 * S

            def body():
                if mode == "dma":
                    _dma_body()
                    return

                # ---- occupancy: keys -> per-quarter int16 idx -> scatter
                # (all 32 instances as 32 GPSIMD channels) ----
                pv = ptsb[:].rearrange("p (h f) -> p h f", h=2)
                keys = spool.tile([NI, NPTS], I32)
                kx = spool.tile([NI, NPTS], I32)
                # keys = ((y >> 4) << 6) + (x >> 4)
                nc.vector.tensor_scalar(
                    keys[:],
                    pv[:, 0, :],
                    4,
                    6,
                    op0=mybir.AluOpType.logical_shift_right,
                    op1=mybir.AluOpType.logical_shift_left,
                )
                nc.vector.tensor_scalar(
                    kx[:], pv[:, 1, :], 4, None,
                    op0=mybir.AluOpType.logical_shift_right,
                )
                nc.vector.tensor_tensor(
                    keys[:], keys[:], kx[:], op=mybir.AluOpType.add
                )

                ones32 = spool.tile([NI, NPTS], BF16)
                nc.gpsimd.memset(ones32[:], 1.0)
                occ = spool.tile([NI, NCELL], BF16)

                # all 4 quarters' index prep upfront (distinct tiles, so the
                # DVE work never serializes behind the GPSIMD scatters)
                idx16s = []
                for q in range(4):
                    t = spool.tile([NI, NPTS], I32, tag=f"tq{q}")
                    ge = spool.tile([NI, NPTS], I32, tag=f"geq{q}")
                    lt = spool.tile([NI, NPTS], I32, tag=f"ltq{q}")
                    nc.vector.tensor_scalar(
                        t[:], keys[:], 1024 * q, None,
                        op0=mybir.AluOpType.subtract,
                    )
                    nc.vector.tensor_scalar(
                        ge[:], t[:], 0, None, op0=mybir.AluOpType.is_ge
                    )
                    nc.vector.tensor_scalar(
                        lt[:], t[:], 1024, None, op0=mybir.AluOpType.is_lt
                    )
                    nc.vector.tensor_tensor(
                        ge[:], ge[:], lt[:], op=mybir.AluOpType.mult
                    )
                    # idx = t + (m * 8192 - 8192): negative outside range
                    nc.vector.tensor_scalar(
                        ge[:], ge[:], 8192, -8192,
                        op0=mybir.AluOpType.mult, op1=mybir.AluOpType.add,
                    )
                    nc.vector.tensor_tensor(
                        t[:], t[:], ge[:], op=mybir.AluOpType.add
                    )
                    idx16 = spool.tile([NI, NPTS], I16, tag=f"idxq{q}")
                    nc.vector.tensor_copy(idx16[:], t[:])
                    idx16s.append(idx16)
                for q in range(4):
                    nc.gpsimd.local_scatter(
                        out_ap=occ[:, q * 1024 : (q + 1) * 1024],
                        data_ap=ones32[:],
                        idxs_ap=idx16s[q][:],
                        channels=NI,
                        num_elems=1024,
                        num_idxs=NPTS,
                    )

                # ---- main pipeline over tapered cell chunks ----
                m1 = m1pool.tile([66, NCELL], F32)
                pst_all = pst.tile([128, 2 * NCK], F32)
                va = spool.tile([128, 2 * NCK], F32)
                sg = spool.tile([128, NCK], F32)
                ff = spool.tile([128, NCK], F32)
                psum3 = ps3.tile([NI, 40], F32)
                occt = cpool.tile([128, NCK * NI], F32)
                vav = va[:].rearrange("p (c two) -> p c two", two=2)

                ats = []
                chunk_sub0 = []  # first global sub index of each chunk
                sub_bases = []

                def emit_chain(cc):
                    """Post-GEMM chain for chunk cc: vl/z transpose, cell
                    gate f=sigmoid(z), scores exp, at-tile assembly. The
                    reference's exp(score + ln(sig+eps)) is computed as
                    exp(score)*sigmoid(z) (the eps term is a <=1e-6 additive
                    perturbation of the softmax weights). Emitted one chunk
                    behind the GEMM so PE never waits on the ACT/DVE chain."""
                    jj0 = chunk_sub0[cc]
                    nsub = (CHUNKS[cc]) // 128
                    js = slice(jj0, jj0 + nsub)
                    for s in range(nsub):
                        sj = jj0 + s
                        cs = slice(sj * 128, (sj + 1) * 128)
                        nc.tensor.matmul(
                            pst_all[:, 2 * sj : 2 * sj + 2],
                            m1[64:66, cs],
                            id128[64:66, 64:66],
                            is_transpose=True,
                        )
                    pstv = pst_all[:].rearrange("p (c two) -> p c two", two=2)
                    # vl to SBUF (for the num-column mult); z is consumed
                    # straight from PSUM by the exp below
                    nc.vector.tensor_copy(vav[:, js, 0], pstv[:, js, 0])
                    # f = sigmoid(z) = 1/(1+e^-z)
                    nc.scalar.activation(
                        sg[:, js], pstv[:, js, 1],
                        mybir.ActivationFunctionType.Exp, scale=-1.0,
                    )
                    nc.vector.tensor_scalar(
                        sg[:, js], sg[:, js], 1.0, None, op0=mybir.AluOpType.add
                    )
                    nc.vector.reciprocal(ff[:, js], sg[:, js])
                    # scores for the whole chunk in one psum bank, one exp
                    psum2 = ps2.tile([128, 4 * NCAPS], F32, tag="ps2")
                    for s in range(nsub):
                        sj = jj0 + s
                        cs = slice(sj * 128, (sj + 1) * 128)
                        nc.tensor.matmul(
                            psum2[:, s * NCAPS : (s + 1) * NCAPS],
                            m1[0:64, cs],
                            qsb[:],
                        )
                    at0 = apool.tile([128, 4 * NCAPS], F32, tag="at0")
                    nc.scalar.activation(
                        at0[:, 0 : nsub * NCAPS],
                        psum2[:, 0 : nsub * NCAPS],
                        mybir.ActivationFunctionType.Exp,
                    )
                    # A-tiles; at cols [den 0:19|cnt 19|num 20:39|corr 39]
                    for s in range(nsub):
                        sj = jj0 + s
                        at = apool.tile([128, 40], F32, tag="at")
                        nc.vector.tensor_scalar(
                            at[:, 0:NCAPS],
                            at0[:, s * NCAPS : (s + 1) * NCAPS],
                            ff[:, sj : sj + 1],
                            None,
                            op0=mybir.AluOpType.mult,
                        )
                        nc.vector.tensor_scalar(
                            at[:, 20:39],
                            at[:, 0:NCAPS],
                            va[:, 2 * sj : 2 * sj + 1],
                            None,
                            op0=mybir.AluOpType.mult,
                        )
                        atv = at[:].rearrange("p (b r) -> p r b", b=2)
                        wv = wcsb[:].rearrange("p (j two) -> p j two", two=2)
                        nc.vector.tensor_copy(
                            atv[:, 19:20, :], wv[:, sj : sj + 1, :]
                        )
                        ats.append(at)

                ps3_done = 0

                def drain_ps3(upto):
                    nonlocal ps3_done
                    while ps3_done < min(upto, len(ats)):
                        p = ps3_done
                        nc.tensor.matmul(
                            psum3[:],
                            occt[:, p * NI : (p + 1) * NI],
                            ats[p][:],
                            start=(p == 0),
                            stop=(p == NCK - 1),
                        )
                        ps3_done += 1

                col = 0
                base = 0
                jj = 0
                for c, S in enumerate(CHUNKS):
                    chunk_sub0.append(jj)
                    sub_bases.append(base)
                    if mode != "compute":
                        xt = xpool.tile([128, 5120], F16, tag="xt")
                        nc.sync.dma_start(
                            xt[:, 0 : 10 * S], X16[:, col : col + 10 * S]
                        )
                    else:
                        xt = xres
                    if c == 0:
                        _small_dmas()
                    psum = ps1.tile([66, 512], F32, tag="ps1")
                    for k in range(NCH):
                        nc.tensor.matmul(
                            psum[:, 0:S],
                            wsb[:, k * 66 : (k + 1) * 66],
                            xt[:, k * S : (k + 1) * S],
                            start=(k == 0),
                            stop=False,
                        )
                    nc.tensor.matmul(
                        psum[:, 0:S],
                        wsb[0:3, 660:726],
                        c3sb[:, base : base + S],
                        start=False,
                        stop=True,
                    )
                    nc.scalar.copy(m1[:, base : base + S], psum[:, 0:S])
                    jj += S // 128
                    base += S
                    col += 10 * S

                    if c >= 1:
                        emit_chain(c - 1)
                    if c == PSO_AFTER:
                        # occ -> occt [128 cells, 32 inst] per subchunk
                        pso_all = pso.tile([128, NCK * NI], BF16)
                        for q32 in range(NCK):
                            nc.tensor.matmul(
                                pso_all[:, q32 * NI : (q32 + 1) * NI],
                                occ[:, q32 * 128 : (q32 + 1) * 128],
                                id32b[:],
                                is_transpose=True,
                            )
                        nc.vector.tensor_copy(occt[:], pso_all[:])
                    if c >= PS3_AFTER:
                        # consume at-tiles two chunks behind the GEMM
                        drain_ps3(chunk_sub0[c - 1])
                emit_chain(len(CHUNKS) - 1)
                drain_ps3(NCK)

                # ---- finalize: sigmoid(num/den + corr/n) ----
                rc = spool.tile([NI, 20], F32)
                tt = spool.tile([NI, 20], F32)
                nc.vector.reciprocal(rc[:], psum3[:, 0:20])
                nc.vector.tensor_tensor(
                    tt[:], psum3[:, 20:40], rc[:], op=mybir.AluOpType.mult
                )
                t1 = spool.tile([NI, NCAPS], F32)
                nc.vector.tensor_scalar(
                    t1[:], tt[:, 0:NCAPS], tt[:, 19:20], None,
                    op0=mybir.AluOpType.add,
                )
                # sigmoid(L) = 1/(1+exp(-L)): one ACT exp + two DVE ops
                osb = spool.tile([NI, NCAPS], F32)
                nc.scalar.activation(
                    osb[:], t1[:], mybir.ActivationFunctionType.Exp, scale=-1.0
                )
                nc.vector.tensor_scalar(
                    osb[:], osb[:], 1.0, None, op0=mybir.AluOpType.add
                )
                nc.vector.reciprocal(osb[:], osb[:])
                nc.sync.dma_start(OUT[:], osb[:])

            if loop_n == 1:
                body()
            else:
                with tc.For_i(0, loop_n, 1):
                    body()

    nc.compile()
    _CACHE[key] = nc
    return nc


def _fold_weights(Wp, bp, Wa, ba, Q, Wk, bk, Wv, bv, Wl, bl):
    f = lambda t: np.asarray(t, np.float64)
    Wp, bp, Wa, ba, Q, Wk, bk, Wv, bv, Wl, bl = map(
        f, (Wp, bp, Wa, ba, Q, Wk, bk, Wv, bv, Wl, bl)
    )
    wl = Wl[:, 0]
    WK = Wp.T @ Wk[:256]
    wvl_cap = Wv[:256] @ wl
    a, b = Wv[256] @ wl, Wv[257] @ wl

    W_all = np.zeros((CIN + 3, 66), np.float64)
    W_all[:CIN, :64] = WK
    W_all[:CIN, 64] = Wp.T @ wvl_cap
    W_all[:CIN, 65] = Wa[0]
    W_all[CIN + 0, :64] = Wk[256] / 64.0
    W_all[CIN + 1, :64] = Wk[257] / 64.0
    W_all[CIN + 2, :64] = bp @ Wk[:256] + bk
    W_all[CIN + 0, 64] = a / 64.0
    W_all[CIN + 1, 64] = b / 64.0
    W_all[CIN + 2, 64] = bp @ wvl_cap + bv @ wl
    W_all[CIN + 2, 65] = ba[0]

    # SBUF layout: [128, 726] fp16, channel chunk k at cols 66k..66k+66,
    # const rows (y/64, x/64, bias) at cols 660:726 on partitions 0:3.
    W16 = np.zeros((128, 726), np.float16)
    for k in range(NCH):
        W16[:, k * 66 : (k + 1) * 66] = W_all[k * 128 : (k + 1) * 128].astype(
            np.float16
        )
    W16[0:3, 660:726] = W_all[CIN : CIN + 3].astype(np.float16)

    c = np.arange(NCELL)
    y64 = (c // 64) / 64.0
    x64 = (c % 64) / 64.0
    wcorr = -(a * y64 + b * x64 - bl[0])
    WC2 = np.empty((128, 2 * NCK), np.float64)
    WC2[:, 0::2] = 1.0  # count column of the at-tile
    WC2[:, 1::2] = wcorr.reshape(NCK, 128).T
    return W16, (Q.T / 8.0).astype(np.float32), WC2.astype(np.float32)


def _pack_x16(fo):
    """[B, CIN, 64, 64] f32 -> [B, 128, XCOLS] fp16 packed per chunk:
    chunk c of size S at cell base -> cols 10*base .. 10*base+10*S laid out
    as k-major [128, 10*S]."""
    Bn = fo.shape[0]
    X = np.asarray(fo, np.float32).reshape(Bn, NCH, 128, NCELL).astype(np.float16)
    out = np.empty((Bn, 128, XCOLS), np.float16)
    col = 0
    base = 0
    for S in CHUNKS:
        blk = X[:, :, :, base : base + S]  # [B, 10, 128, S]
        out[:, :, col : col + 10 * S] = blk.transpose(0, 2, 1, 3).reshape(
            Bn, 128, 10 * S
        )
        col += 10 * S
        base += S
    return out


def _make_in_maps(
    feature_output, Wp, bp, Wa, ba, Q, Wk, bk, Wv, bv, Wl, bl, point_lists
):
    W16, QT8, WC2 = _fold_weights(Wp, bp, Wa, ba, Q, Wk, bk, Wv, bv, Wl, bl)

    c = np.arange(NCELL)
    C316 = np.stack([c // 64, c % 64, np.ones(NCELL)]).astype(np.float16)

    X16 = _pack_x16(np.asarray(feature_output, np.float32))
    pts = np.ascontiguousarray(np.asarray(point_lists).astype(np.int32)).reshape(
        B, NI, 2 * NPTS
    )

    return [
        {
            "X16": X16[i],
            "W16": W16,
            "QT8": QT8,
            "WC2": WC2,
            "PTS": pts[i],
            "C316": C316,
        }
        for i in range(B)
    ]


def kernel(
    feature_output, Wp, bp, Wa, ba, Q, Wk, bk, Wv, bv, Wl, bl, point_lists
):
    nc = _build_nc()
    in_maps = _make_in_maps(
        feature_output, Wp, bp, Wa, ba, Q, Wk, bk, Wv, bv, Wl, bl, point_lists
    )
    res = run_bass_kernel_spmd(nc, in_maps, core_ids=list(range(B)))
    return np.stack([res.results[i]["OUT"] for i in range(B)]).astype(np.float32)


# revision 26
# speedup vs baseline: 1.0610x; 1.0610x over previous
"""Capsule-routing kernel for Trainium2, data-parallel over batch (8 cores).

Math: the reference's per-instance routing (unique -> gather -> attention)
is reformulated as a dense masked softmax over the 64x64 cell grid:
  - all per-cell quantities (attention keys, value-scalar, activation logit)
    come from one fused per-image GEMM,
  - the relative-position encoding's mean term cancels in the softmax and
    reduces to a rank-1 correction computed from per-instance occupancy sums,
  - per-instance dedup of points is a scatter of ones into a cell bitmap,
  - all 32 instances reduce in a single accumulated PE matmul against the
    occupancy mask.

Perf structure (v2):
  - X is converted to fp16 host-side and packed so each cell-chunk's ten
    128-channel slices are one contiguous [128, 10*S] DMA (halves HBM
    traffic, the roofline term, and needs one HWDGE op per chunk),
  - chunk sizes taper [512x7, 256, 128, 128] so the post-GEMM chain of the
    last chunk (the serial tail after the final X byte) is short,
  - all small input DMAs ride the ACT HWDGE ring, X rides the SP ring,
    nothing serializes on GPSIMD descriptor generation,
  - the occupancy scatter pipeline is emitted first so occt is ready by the
    time PE reaches the deferred first psum3 accumulation block,
  - at-tiles pool is deep (36) so psum3 consumption can never backpressure
    the X stream.
"""
import sys

sys.path.insert(0, "/opt/trn_rl_repo")

import numpy as np

import concourse.bacc as bacc
import concourse.mybir as mybir
from concourse import masks, tile
from concourse.bass_utils import run_bass_kernel_spmd

F32 = mybir.dt.float32
F16 = mybir.dt.float16
BF16 = mybir.dt.bfloat16
I32 = mybir.dt.int32
I16 = mybir.dt.int16

B = 8
CIN = 1280
NCELL = 4096  # 64x64 feature grid
NCAPS = 19
NI = 32  # instances per image
NPTS = 256  # points per instance
DK = 64
EPS = 1e-6
NCH = 10  # channel chunks of 128
NCK = 32  # 128-cell subchunks
CHUNKS = [128, 384] + [512] * 6 + [384, 128]  # cell chunks, sum = 4096
XCOLS = 10 * NCELL  # packed X16 columns
PSO_AFTER = 3  # emit occ transposes after this chunk's GEMM
PS3_AFTER = 4  # start draining psum3 accumulation backlog at this chunk

_CACHE = {}

# Force every activation onto the one table set that covers exp/ln/copy so
# the ACT engine never reloads its function tables mid-kernel.
_ONE_SET = "natural_log_exp_and_others"
_orig_get_tables = None


def _patched_tables(arch):
    full = _orig_get_tables(arch)
    return {
        name: (funcs if name == _ONE_SET else set())
        for name, funcs in full.items()
    }


def _install_act_table_patch():
    global _orig_get_tables
    if _orig_get_tables is None:
        _orig_get_tables = bacc.get_activation_tables
        bacc.get_activation_tables = _patched_tables


def _build_nc(dbg=False, loop_n=1, mode="full"):
    key = ("nc", dbg, loop_n, mode)
    if key in _CACHE:
        return _CACHE[key]

    _install_act_table_patch()
    nc = bacc.Bacc(None, target_bir_lowering=False, debug=False)

    X16 = nc.dram_tensor("X16", [128, XCOLS], F16, kind="ExternalInput")
    W16 = nc.dram_tensor("W16", [128, 726], F16, kind="ExternalInput")
    QT8 = nc.dram_tensor("QT8", [DK, NCAPS], F32, kind="ExternalInput")
    WC2 = nc.dram_tensor("WC2", [128, 2 * NCK], F32, kind="ExternalInput")
    PTS = nc.dram_tensor("PTS", [NI, 2 * NPTS], I32, kind="ExternalInput")
    C316 = nc.dram_tensor("C316", [3, NCELL], F16, kind="ExternalInput")
    OUT = nc.dram_tensor("OUT", [NI, NCAPS], F32, kind="ExternalOutput")

    with tile.TileContext(nc) as tc:
        with (
            tc.tile_pool(name="const", bufs=1) as cpool,
            tc.tile_pool(name="xp", bufs=6) as xpool,
            tc.tile_pool(name="m1", bufs=1) as m1pool,
            tc.tile_pool(name="small", bufs=1) as spool,
            tc.tile_pool(name="ap", bufs=36) as apool,
            tc.tile_pool(name="ps1", bufs=3, space="PSUM") as ps1,
            tc.tile_pool(name="pst", bufs=1, space="PSUM") as pst,
            tc.tile_pool(name="ps2", bufs=2, space="PSUM") as ps2,
            tc.tile_pool(name="pso", bufs=1, space="PSUM") as pso,
            tc.tile_pool(name="ps3", bufs=1, space="PSUM") as ps3,
        ):
            # ---- constants ----
            id128 = cpool.tile([128, 128], F32)
            masks.make_identity(nc, id128[:])
            id32b = cpool.tile([32, 32], BF16)
            masks.make_identity(nc, id32b[:])

            # ---- small input DMAs (SP HWDGE ring). ptsb/wsb go ahead of
            # the X stream (needed first); the rest are issued inside the
            # chunk loop after X chunk 0 so their HWDGE gen hides under it.
            ptsb = spool.tile([NI, 2 * NPTS], I32)
            nc.scalar.dma_start(ptsb[:], PTS[:])  # ACT ring, parallel HWDGE
            wsb = cpool.tile([128, 726], F16)
            nc.sync.dma_start(wsb[:], W16[:])
            qsb = cpool.tile([DK, NCAPS], F32)
            wcsb = cpool.tile([128, 2 * NCK], F32)
            c3sb = cpool.tile([3, NCELL], F16)

            def _small_dmas():
                nc.sync.dma_start(c3sb[:], C316[:])
                nc.sync.dma_start(qsb[:], QT8[:])
                nc.sync.dma_start(wcsb[:], WC2[:])

            xres = cpool.tile([128, 5120], F16)
            if mode == "compute":
                nc.sync.dma_start(xres[:], X16[:, 0:5120])

            def _dma_body():
                col = 0
                for S in CHUNKS:
                    xt = xpool.tile([128, 5120], F16, tag="xt")
                    nc.sync.dma_start(xt[:, 0 : 10 * S], X16[:, col : col + 10 * S])
                    col += 10 * S

            def body():
                if mode == "dma":
                    _dma_body()
                    return

                # ---- occupancy: keys -> per-quarter int16 idx -> scatter
                # (all 32 instances as 32 GPSIMD channels) ----
                pv = ptsb[:].rearrange("p (h f) -> p h f", h=2)
                keys = spool.tile([NI, NPTS], I32)
                kx = spool.tile([NI, NPTS], I32)
                # keys = ((y >> 4) << 6) + (x >> 4)
                nc.vector.tensor_scalar(
                    keys[:],
                    pv[:, 0, :],
                    4,
                    6,
                    op0=mybir.AluOpType.logical_shift_right,
                    op1=mybir.AluOpType.logical_shift_left,
                )
                nc.vector.tensor_scalar(
                    kx[:], pv[:, 1, :], 4, None,
                    op0=mybir.AluOpType.logical_shift_right,
                )
                nc.vector.tensor_tensor(
                    keys[:], keys[:], kx[:], op=mybir.AluOpType.add
                )

                ones32 = spool.tile([NI, NPTS], BF16)
                nc.gpsimd.memset(ones32[:], 1.0)
                occ = spool.tile([NI, NCELL], BF16)

                # all 4 quarters' index prep upfront (distinct tiles, so the
                # DVE work never serializes behind the GPSIMD scatters)
                idx16s = []
                for q in range(4):
                    t = spool.tile([NI, NPTS], I32, tag=f"tq{q}")
                    ge = spool.tile([NI, NPTS], I32, tag=f"geq{q}")
                    lt = spool.tile([NI, NPTS], I32, tag=f"ltq{q}")
                    nc.vector.tensor_scalar(
                        t[:], keys[:], 1024 * q, None,
                        op0=mybir.AluOpType.subtract,
                    )
                    nc.vector.tensor_scalar(
                        ge[:], t[:], 0, None, op0=mybir.AluOpType.is_ge
                    )
                    nc.vector.tensor_scalar(
                        lt[:], t[:], 1024, None, op0=mybir.AluOpType.is_lt
                    )
                    nc.vector.tensor_tensor(
                        ge[:], ge[:], lt[:], op=mybir.AluOpType.mult
                    )
                    # idx = t + (m * 8192 - 8192): negative outside range
                    nc.vector.tensor_scalar(
                        ge[:], ge[:], 8192, -8192,
                        op0=mybir.AluOpType.mult, op1=mybir.AluOpType.add,
                    )
                    nc.vector.tensor_tensor(
                        t[:], t[:], ge[:], op=mybir.AluOpType.add
                    )
                    idx16 = spool.tile([NI, NPTS], I16, tag=f"idxq{q}")
                    nc.vector.tensor_copy(idx16[:], t[:])
                    idx16s.append(idx16)
                for q in range(4):
                    nc.gpsimd.local_scatter(
                        out_ap=occ[:, q * 1024 : (q + 1) * 1024],
                        data_ap=ones32[:],
                        idxs_ap=idx16s[q][:],
                        channels=NI,
                        num_elems=1024,
                        num_idxs=NPTS,
                    )

                # ---- main pipeline over tapered cell chunks ----
                m1 = m1pool.tile([66, NCELL], F32)
                pst_all = pst.tile([128, 2 * NCK], F32)
                va = spool.tile([128, 2 * NCK], F32)
                sg = spool.tile([128, NCK], F32)
                ff = spool.tile([128, NCK], F32)
                psum3 = ps3.tile([NI, 40], F32)
                occt = cpool.tile([128, NCK * NI], F32)
                vav = va[:].rearrange("p (c two) -> p c two", two=2)

                ats = []
                chunk_sub0 = []  # first global sub index of each chunk
                sub_bases = []

                def emit_chain(cc):
                    """Post-GEMM chain for chunk cc: vl/z transpose, cell
                    gate f=sigmoid(z), scores exp, at-tile assembly. The
                    reference's exp(score + ln(sig+eps)) is computed as
                    exp(score)*sigmoid(z) (the eps term is a <=1e-6 additive
                    perturbation of the softmax weights). Emitted one chunk
                    behind the GEMM so PE never waits on the ACT/DVE chain."""
                    jj0 = chunk_sub0[cc]
                    nsub = (CHUNKS[cc]) // 128
                    js = slice(jj0, jj0 + nsub)
                    for s in range(nsub):
                        sj = jj0 + s
                        cs = slice(sj * 128, (sj + 1) * 128)
                        nc.tensor.matmul(
                            pst_all[:, 2 * sj : 2 * sj + 2],
                            m1[64:66, cs],
                            id128[64:66, 64:66],
                            is_transpose=True,
                        )
                    pstv = pst_all[:].rearrange("p (c two) -> p c two", two=2)
                    # vl to SBUF (for the num-column mult); z is consumed
                    # straight from PSUM by the exp below
                    nc.vector.tensor_copy(vav[:, js, 0], pstv[:, js, 0])
                    # f = sigmoid(z) = 1/(1+e^-z)
                    nc.scalar.activation(
                        sg[:, js], pstv[:, js, 1],
                        mybir.ActivationFunctionType.Exp, scale=-1.0,
                    )
                    nc.vector.tensor_scalar(
                        sg[:, js], sg[:, js], 1.0, None, op0=mybir.AluOpType.add
                    )
                    nc.vector.reciprocal(ff[:, js], sg[:, js])
                    # scores for the whole chunk in one psum bank, one exp
                    psum2 = ps2.tile([128, 4 * NCAPS], F32, tag="ps2")
                    for s in range(nsub):
                        sj = jj0 + s
                        cs = slice(sj * 128, (sj + 1) * 128)
                        nc.tensor.matmul(
                            psum2[:, s * NCAPS : (s + 1) * NCAPS],
                            m1[0:64, cs],
                            qsb[:],
                        )
                    at0 = apool.tile([128, 4 * NCAPS], F32, tag="at0")
                    nc.scalar.activation(
                        at0[:, 0 : nsub * NCAPS],
                        psum2[:, 0 : nsub * NCAPS],
                        mybir.ActivationFunctionType.Exp,
                    )
                    # A-tiles; at cols [den 0:19|cnt 19|num 20:39|corr 39]
                    for s in range(nsub):
                        sj = jj0 + s
                        at = apool.tile([128, 40], F32, tag="at")
                        nc.vector.tensor_scalar(
                            at[:, 0:NCAPS],
                            at0[:, s * NCAPS : (s + 1) * NCAPS],
                            ff[:, sj : sj + 1],
                            None,
                            op0=mybir.AluOpType.mult,
                        )
                        nc.vector.tensor_scalar(
                            at[:, 20:39],
                            at[:, 0:NCAPS],
                            va[:, 2 * sj : 2 * sj + 1],
                            None,
                            op0=mybir.AluOpType.mult,
                        )
                        atv = at[:].rearrange("p (b r) -> p r b", b=2)
                        wv = wcsb[:].rearrange("p (j two) -> p j two", two=2)
                        nc.vector.tensor_copy(
                            atv[:, 19:20, :], wv[:, sj : sj + 1, :]
                        )
                        ats.append(at)

                ps3_done = 0

                def drain_ps3(upto):
                    nonlocal ps3_done
                    while ps3_done < min(upto, len(ats)):
                        p = ps3_done
                        nc.tensor.matmul(
                            psum3[:],
                            occt[:, p * NI : (p + 1) * NI],
                            ats[p][:],
                            start=(p == 0),
                            stop=(p == NCK - 1),
                        )
                        ps3_done += 1

                col = 0
                base = 0
                jj = 0
                for c, S in enumerate(CHUNKS):
                    chunk_sub0.append(jj)
                    sub_bases.append(base)
                    if mode != "compute":
                        xt = xpool.tile([128, 5120], F16, tag="xt")
                        nc.sync.dma_start(
                            xt[:, 0 : 10 * S], X16[:, col : col + 10 * S]
                        )
                    else:
                        xt = xres
                    if c == 0:
                        _small_dmas()
                    psum = ps1.tile([66, 512], F32, tag="ps1")
                    for k in range(NCH):
                        nc.tensor.matmul(
                            psum[:, 0:S],
                            wsb[:, k * 66 : (k + 1) * 66],
                            xt[:, k * S : (k + 1) * S],
                            start=(k == 0),
                            stop=False,
                        )
                    nc.tensor.matmul(
                        psum[:, 0:S],
                        wsb[0:3, 660:726],
                        c3sb[:, base : base + S],
                        start=False,
                        stop=True,
                    )
                    nc.scalar.copy(m1[:, base : base + S], psum[:, 0:S])
                    jj += S // 128
                    base += S
                    col += 10 * S

                    if c >= 1:
                        emit_chain(c - 1)
                    if c == PSO_AFTER:
                        # occ -> occt [128 cells, 32 inst] per subchunk
                        pso_all = pso.tile([128, NCK * NI], BF16)
                        for q32 in range(NCK):
                            nc.tensor.matmul(
                                pso_all[:, q32 * NI : (q32 + 1) * NI],
                                occ[:, q32 * 128 : (q32 + 1) * 128],
                                id32b[:],
                                is_transpose=True,
                            )
                        nc.vector.tensor_copy(occt[:], pso_all[:])
                    if c >= PS3_AFTER:
                        # consume at-tiles two chunks behind the GEMM
                        drain_ps3(chunk_sub0[c - 1])
                emit_chain(len(CHUNKS) - 1)
                drain_ps3(NCK)

                # ---- finalize: sigmoid(num/den + corr/n) ----
                rc = spool.tile([NI, 20], F32)
                tt = spool.tile([NI, 20], F32)
                nc.vector.reciprocal(rc[:], psum3[:, 0:20])
                nc.vector.tensor_tensor(
                    tt[:], psum3[:, 20:40], rc[:], op=mybir.AluOpType.mult
                )
                t1 = spool.tile([NI, NCAPS], F32)
                nc.vector.tensor_scalar(
                    t1[:], tt[:, 0:NCAPS], tt[:, 19:20], None,
                    op0=mybir.AluOpType.add,
                )
                # sigmoid(L) = 1/(1+exp(-L)): one ACT exp + two DVE ops
                osb = spool.tile([NI, NCAPS], F32)
                nc.scalar.activation(
                    osb[:], t1[:], mybir.ActivationFunctionType.Exp, scale=-1.0
                )
                nc.vector.tensor_scalar(
                    osb[:], osb[:], 1.0, None, op0=mybir.AluOpType.add
                )
                nc.vector.reciprocal(osb[:], osb[:])
                nc.sync.dma_start(OUT[:], osb[:])

            if loop_n == 1:
                body()
            else:
                with tc.For_i(0, loop_n, 1):
                    body()

    nc.compile()
    _CACHE[key] = nc
    return nc


def _fold_weights(Wp, bp, Wa, ba, Q, Wk, bk, Wv, bv, Wl, bl):
    f = lambda t: np.asarray(t, np.float64)
    Wp, bp, Wa, ba, Q, Wk, bk, Wv, bv, Wl, bl = map(
        f, (Wp, bp, Wa, ba, Q, Wk, bk, Wv, bv, Wl, bl)
    )
    wl = Wl[:, 0]
    WK = Wp.T @ Wk[:256]
    wvl_cap = Wv[:256] @ wl
    a, b = Wv[256] @ wl, Wv[257] @ wl

    W_all = np.zeros((CIN + 3, 66), np.float64)
    W_all[:CIN, :64] = WK
    W_all[:CIN, 64] = Wp.T @ wvl_cap
    W_all[:CIN, 65] = Wa[0]
    W_all[CIN + 0, :64] = Wk[256] / 64.0
    W_all[CIN + 1, :64] = Wk[257] / 64.0
    W_all[CIN + 2, :64] = bp @ Wk[:256] + bk
    W_all[CIN + 0, 64] = a / 64.0
    W_all[CIN + 1, 64] = b / 64.0
    W_all[CIN + 2, 64] = bp @ wvl_cap + bv @ wl
    W_all[CIN + 2, 65] = ba[0]

    # SBUF layout: [128, 726] fp16, channel chunk k at cols 66k..66k+66,
    # const rows (y/64, x/64, bias) at cols 660:726 on partitions 0:3.
    W16 = np.zeros((128, 726), np.float16)
    for k in range(NCH):
        W16[:, k * 66 : (k + 1) * 66] = W_all[k * 128 : (k + 1) * 128].astype(
            np.float16
        )
    W16[0:3, 660:726] = W_all[CIN : CIN + 3].astype(np.float16)

    c = np.arange(NCELL)
    y64 = (c // 64) / 64.0
    x64 = (c % 64) / 64.0
    wcorr = -(a * y64 + b * x64 - bl[0])
    WC2 = np.empty((128, 2 * NCK), np.float64)
    WC2[:, 0::2] = 1.0  # count column of the at-tile
    WC2[:, 1::2] = wcorr.reshape(NCK, 128).T
    return W16, (Q.T / 8.0).astype(np.float32), WC2.astype(np.float32)


def _pack_x16(fo):
    """[B, CIN, 64, 64] f32 -> [B, 128, XCOLS] fp16 packed per chunk:
    chunk c of size S at cell base -> cols 10*base .. 10*base+10*S laid out
    as k-major [128, 10*S]."""
    Bn = fo.shape[0]
    X = np.asarray(fo, np.float32).reshape(Bn, NCH, 128, NCELL).astype(np.float16)
    out = np.empty((Bn, 128, XCOLS), np.float16)
    col = 0
    base = 0
    for S in CHUNKS:
        blk = X[:, :, :, base : base + S]  # [B, 10, 128, S]
        out[:, :, col : col + 10 * S] = blk.transpose(0, 2, 1, 3).reshape(
            Bn, 128, 10 * S
        )
        col += 10 * S
        base += S
    return out


def _make_in_maps(
    feature_output, Wp, bp, Wa, ba, Q, Wk, bk, Wv, bv, Wl, bl, point_lists
):
    W16, QT8, WC2 = _fold_weights(Wp, bp, Wa, ba, Q, Wk, bk, Wv, bv, Wl, bl)

    c = np.arange(NCELL)
    C316 = np.stack([c // 64, c % 64, np.ones(NCELL)]).astype(np.float16)

    X16 = _pack_x16(np.asarray(feature_output, np.float32))
    pts = np.ascontiguousarray(np.asarray(point_lists).astype(np.int32)).reshape(
        B, NI, 2 * NPTS
    )

    return [
        {
            "X16": X16[i],
            "W16": W16,
            "QT8": QT8,
            "WC2": WC2,
            "PTS": pts[i],
            "C316": C316,
        }
        for i in range(B)
    ]


def kernel(
    feature_output, Wp, bp, Wa, ba, Q, Wk, bk, Wv, bv, Wl, bl, point_lists
):
    nc = _build_nc()
    in_maps = _make_in_maps(
        feature_output, Wp, bp, Wa, ba, Q, Wk, bk, Wv, bv, Wl, bl, point_lists
    )
    res = run_bass_kernel_spmd(nc, in_maps, core_ids=list(range(B)))
    return np.stack([res.results[i]["OUT"] for i in range(B)]).astype(np.float32)
